# revision 2
# baseline (speedup 1.0000x reference)
"""Trainium2 Bass kernel for nn_AttentionHead_6786048328376.

8-head spatial attention block: q/k/v 1x1-conv projections with additive
positional embedding on q/k, softmax over the QUERY axis (dim=2), attention
apply, channel-major output, 2-layer 1x1-conv MLP with mish, residual add.

Sharding: pure data-parallel over batch — 8 batch elements, one per
NeuronCore. Weights are replicated; no collectives.

Per-core layout choices (C=512, N=H*W=1024, 8 heads, dh=32, ch=64):
  - x is held channel-major [512, 1024]; q/k land head-stacked [256, 1024]
    (row = 32*head + d) so each 128-partition tile holds 4 heads on the four
    32-row PE array strips -> scores use 4-way row-tiled K=32 matmuls.
  - scores are computed TRANSPOSED: sT[m, n] (key-major) so the softmax
    reduction over the query axis n is a free-axis reduction; exp runs on the
    scalar engine with fused row-sum accumulation (no max subtraction needed:
    |scores| <= ~8 for this problem scale).
  - v is computed directly transposed vT[n, c] = x.T @ Wv.T; the softmax
    1/sum is folded into vT rows (64 els/row) instead of dividing the 1M-el
    score matrix.
  - attention apply uses 2-way col-tiled matmuls (two heads concurrent,
    M=64 each) producing attn channel-major [512, 1024] with no transposes.
  - mish(x) = x*tanh(ln(1+exp(x))) via Exp -> Ln(bias=1) -> Tanh on ACT plus
    one fused (psum+b1)*tanh vector op.
"""

import numpy as np

_CACHE = {}


def _build():
    import concourse.bacc as bacc
    import concourse.tile as tile
    import concourse.mybir as mybir

    dt = mybir.dt
    F32 = dt.float32
    BF16 = dt.bfloat16
    Act = mybir.ActivationFunctionType
    Alu = mybir.AluOpType

    nc = bacc.Bacc("TRN2", target_bir_lowering=False, debug=False)

    x_d = nc.dram_tensor("x", [512, 1024], F32, kind="ExternalInput").ap()
    wqkt_d = nc.dram_tensor("wqkt", [512, 512], F32, kind="ExternalInput").ap()
    peb_d = nc.dram_tensor("peb", [4, 128, 1024], F32, kind="ExternalInput").ap()
    wvt_d = nc.dram_tensor("wvt", [512, 512], F32, kind="ExternalInput").ap()
    bvb_d = nc.dram_tensor("bvb", [128, 512], F32, kind="ExternalInput").ap()
    w1t_d = nc.dram_tensor("w1t", [512, 512], F32, kind="ExternalInput").ap()
    w2t_d = nc.dram_tensor("w2t", [512, 512], F32, kind="ExternalInput").ap()
    b1_d = nc.dram_tensor("b1", [512, 1], F32, kind="ExternalInput").ap()
    b2_d = nc.dram_tensor("b2", [512, 1], F32, kind="ExternalInput").ap()
    out_d = nc.dram_tensor("out", [512, 1024], F32, kind="ExternalOutput").ap()

    with tile.TileContext(nc) as tc:
        with tc.tile_pool(name="persist", bufs=1) as per, \
             tc.tile_pool(name="mtmp", bufs=18) as mt, \
             tc.tile_pool(name="etp", bufs=16) as etp, \
             tc.tile_pool(name="small", bufs=8) as sm, \
             tc.tile_pool(name="ps", bufs=4, space="PSUM") as ps, \
             tc.tile_pool(name="av", bufs=4, space="PSUM") as av:

            def ptile(shape, dtype, name):
                return per.tile(shape, dtype, name=name, tag=name)

            x_sb = [ptile([128, 1024], F32, f"xsb{i}") for i in range(4)]
            wqk_sb = [ptile([128, 512], F32, f"wqk{i}") for i in range(4)]
            pe_sb = [ptile([128, 1024], F32, f"pe{i}") for i in range(4)]
            wv_sb = [ptile([128, 512], F32, f"wv{i}") for i in range(4)]
            bv_sb = ptile([128, 512], F32, "bvsb")
            w1_sb = [ptile([128, 512], F32, f"w1{i}") for i in range(4)]
            w2_sb = [ptile([128, 512], F32, f"w2{i}") for i in range(4)]
            b1_sb = [ptile([128, 1], F32, f"b1c{i}") for i in range(4)]
            b2_sb = [ptile([128, 1], F32, f"b2c{i}") for i in range(4)]
            qk_sb = [ptile([128, 1024], F32, f"qks{i}") for i in range(4)]
            vt_sb = [ptile([128, 512], F32, f"vts{i}") for i in range(8)]
            attn_sb = [ptile([128, 1024], F32, f"attn{i}") for i in range(4)]
            mish_sb = [ptile([128, 1024], F32, f"mish{i}") for i in range(4)]
            out_sb = [ptile([128, 1024], F32, f"osb{i}") for i in range(4)]

            dma = nc.sync.dma_start
            for i in range(4):
                dma(out=x_sb[i], in_=x_d[128 * i:128 * (i + 1), :])
            for i in range(4):
                dma(out=wqk_sb[i], in_=wqkt_d[128 * i:128 * (i + 1), :])
            for i in range(4):
                dma(out=pe_sb[i], in_=peb_d[i])
            for i in range(4):
                dma(out=wv_sb[i], in_=wvt_d[128 * i:128 * (i + 1), :])
            dma(out=bv_sb, in_=bvb_d)
            for i in range(4):
                dma(out=w1_sb[i], in_=w1t_d[128 * i:128 * (i + 1), :])
                dma(out=w2_sb[i], in_=w2t_d[128 * i:128 * (i + 1), :])
                dma(out=b1_sb[i], in_=b1_d[128 * i:128 * (i + 1), :])
                dma(out=b2_sb[i], in_=b2_d[128 * i:128 * (i + 1), :])

            mm = nc.tensor.matmul

            # q/k projections: qk[512, 1024] = WqkT.T @ x, then + (PE, bias)
            for t in range(4):
                for nh in range(2):
                    pt = ps.tile([128, 512], F32, name="sps", tag="sps")
                    for kc in range(4):
                        mm(pt, lhsT=wqk_sb[kc][:, 128 * t:128 * (t + 1)],
                           rhs=x_sb[kc][:, 512 * nh:512 * (nh + 1)],
                           start=(kc == 0), stop=(kc == 3))
                    nc.vector.tensor_add(
                        qk_sb[t][:, 512 * nh:512 * (nh + 1)], pt,
                        pe_sb[t][:, 512 * nh:512 * (nh + 1)])

            # vT[n, c] = x.T @ WvT, then + bv
            for i in range(8):
                pt = ps.tile([128, 512], F32, name="sps", tag="sps")
                for kc in range(4):
                    mm(pt, lhsT=x_sb[kc][:, 128 * i:128 * (i + 1)], rhs=wv_sb[kc],
                       start=(kc == 0), stop=(kc == 3))
                nc.vector.tensor_add(vt_sb[i], pt, bv_sb)

            # attention: two head-groups of 4 heads
            for g in range(2):
                q_t = qk_sb[g]
                k_t = qk_sb[2 + g]
                av_t = {}
                for p_ in range(2):
                    for nh in range(2):
                        av_t[(p_, nh)] = av.tile([128, 512], F32,
                                                 name="avt", tag="av")
                for mc in range(8):
                    S = sm.tile([128, 8], F32, name="S", tag="S")
                    ets = {}
                    for nh in range(2):
                        for hp in range(4):
                            sp = ps.tile([128, 512], F32, name="sps", tag="sps")
                            mm(sp,
                               lhsT=k_t[32 * hp:32 * (hp + 1),
                                        128 * mc:128 * (mc + 1)],
                               rhs=q_t[32 * hp:32 * (hp + 1),
                                       512 * nh:512 * (nh + 1)],
                               start=True, stop=True,
                               tile_position=(32 * hp, 0))
                            et = etp.tile([128, 512], BF16, name="et", tag="et")
                            c = 4 * nh + hp
                            nc.scalar.activation(et, sp, Act.Exp,
                                                 accum_out=S[:, c:c + 1])
                            ets[(hp, nh)] = et
                    Ss = sm.tile([128, 4], F32, name="Ss", tag="Ss")
                    R = sm.tile([128, 4], F32, name="R", tag="R")
                    nc.vector.tensor_add(Ss, S[:, 0:4], S[:, 4:8])
                    nc.vector.reciprocal(R, Ss)
                    for hp in range(4):
                        h = 4 * g + hp
                        vts = sm.tile([128, 64], BF16, name="vtsc", tag="vtsc")
                        nc.vector.tensor_scalar_mul(
                            vts, vt_sb[mc][:, 64 * h:64 * (h + 1)],
                            R[:, hp:hp + 1])
                        p_, cp = hp // 2, 64 * (hp % 2)
                        for nh in range(2):
                            # two col-tiled accumulation series share each
                            # bank on disjoint partition halves; has_written
                            # is per-element so this is safe — the sim's
                            # coarse zero-region tracker is what we skip.
                            mm(av_t[(p_, nh)][cp:cp + 64, :], lhsT=vts,
                               rhs=ets[(hp, nh)],
                               start=(mc == 0), stop=(mc == 7),
                               tile_position=(0, cp), skip_group_check=True)
                for p_ in range(2):
                    for nh in range(2):
                        nc.vector.tensor_copy(
                            out=attn_sb[2 * g + p_][:, 512 * nh:512 * (nh + 1)],
                            in_=av_t[(p_, nh)])

            # MLP: h1 = W1 @ attn + b1; mish; out = W2 @ mish + b2 + x
            h1ps, t_t, sp_t, th_t = {}, {}, {}, {}
            for nh in range(2):
                pool, tg = (ps, "sps") if nh == 0 else (av, "av")
                for i in range(4):
                    pt = pool.tile([128, 512], F32, name="h1ps", tag=tg)
                    for kc in range(4):
                        mm(pt, lhsT=w1_sb[kc][:, 128 * i:128 * (i + 1)],
                           rhs=attn_sb[kc][:, 512 * nh:512 * (nh + 1)],
                           start=(kc == 0), stop=(kc == 3))
                    h1ps[(nh, i)] = pt
                    tt = mt.tile([128, 512], F32, name="mtt", tag="mtt")
                    nc.scalar.activation(tt, pt, Act.Exp, bias=b1_sb[i])
                    t_t[(nh, i)] = tt
            for key in list(t_t):
                spt = mt.tile([128, 512], F32, name="mtt", tag="mtt")
                nc.scalar.activation(spt, t_t[key], Act.Ln, bias=1.0)
                sp_t[key] = spt
            for key in list(sp_t):
                tht = mt.tile([128, 512], F32, name="mtt", tag="mtt")
                nc.scalar.activation(tht, sp_t[key], Act.Tanh)
                th_t[key] = tht
            for (nh, i), tht in th_t.items():
                nc.vector.scalar_tensor_tensor(
                    out=mish_sb[i][:, 512 * nh:512 * (nh + 1)],
                    in0=h1ps[(nh, i)], scalar=b1_sb[i], in1=tht,
                    op0=Alu.add, op1=Alu.mult)
            for nh in range(2):
                pool, tg = (ps, "sps") if nh == 0 else (av, "av")
                for j in range(4):
                    pt = pool.tile([128, 512], F32, name="h2ps", tag=tg)
                    for kc in range(4):
                        mm(pt, lhsT=w2_sb[kc][:, 128 * j:128 * (j + 1)],
                           rhs=mish_sb[kc][:, 512 * nh:512 * (nh + 1)],
                           start=(kc == 0), stop=(kc == 3))
                    nc.vector.scalar_tensor_tensor(
                        out=out_sb[j][:, 512 * nh:512 * (nh + 1)],
                        in0=pt, scalar=b2_sb[j],
                        in1=x_sb[j][:, 512 * nh:512 * (nh + 1)],
                        op0=Alu.add, op1=Alu.add)
            for j in range(4):
                dma(out=out_d[128 * j:128 * (j + 1), :], in_=out_sb[j])

    nc.compile()
    return nc


def _get_nc():
    if "nc" not in _CACHE:
        _CACHE["nc"] = _build()
    return _CACHE["nc"]


def _make_in_maps(inputs):
    x = np.asarray(inputs["x"], np.float32)
    PE = np.asarray(inputs["PE"], np.float32)
    Wq = np.asarray(inputs["Wq"], np.float32)
    bq = np.asarray(inputs["bq"], np.float32)
    Wk = np.asarray(inputs["Wk"], np.float32)
    bk = np.asarray(inputs["bk"], np.float32)
    Wv = np.asarray(inputs["Wv"], np.float32)
    bv = np.asarray(inputs["bv"], np.float32)
    W1 = np.asarray(inputs["W1"], np.float32)
    b1 = np.asarray(inputs["b1"], np.float32)
    W2 = np.asarray(inputs["W2"], np.float32)
    b2 = np.asarray(inputs["b2"], np.float32)

    s = np.float32(1.0 / np.sqrt(np.float32(32.0)))
    pef = PE.reshape(32, 1024)
    pe4 = np.tile(pef, (4, 1))  # [128, 1024], row = 32*j + d
    peb = np.stack([
        s * (pe4 + bq[0:128][:, None]),
        s * (pe4 + bq[128:256][:, None]),
        pe4 + bk[0:128][:, None],
        pe4 + bk[128:256][:, None],
    ]).astype(np.float32)
    wqkt = np.ascontiguousarray(
        np.concatenate([s * Wq, Wk], axis=0).T.astype(np.float32))
    wvt = np.ascontiguousarray(Wv.T.astype(np.float32))
    bvb = np.ascontiguousarray(
        np.broadcast_to(bv, (128, 512)).astype(np.float32))
    w1t = np.ascontiguousarray(W1.T.astype(np.float32))
    w2t = np.ascontiguousarray(W2.T.astype(np.float32))
    b1c = np.ascontiguousarray(b1.reshape(512, 1).astype(np.float32))
    b2c = np.ascontiguousarray(b2.reshape(512, 1).astype(np.float32))

    xb = np.ascontiguousarray(x.reshape(8, 512, 1024))
    shared = dict(wqkt=wqkt, peb=peb, wvt=wvt, bvb=bvb,
                  w1t=w1t, w2t=w2t, b1=b1c, b2=b2c)
    return [dict(x=np.ascontiguousarray(xb[i]), **shared) for i in range(8)]


def _run(in_maps, trace=False, **kwargs):
    from concourse import bass_utils
    nc = _get_nc()
    return bass_utils.run_bass_kernel_spmd(
        nc, in_maps, core_ids=list(range(8)), trace=trace, **kwargs)


def kernel(**inputs):
    in_maps = _make_in_maps(inputs)
    res = _run(in_maps)
    out = np.stack([r["out"] for r in res.results], axis=0)
    return np.ascontiguousarray(out.reshape(8, 512, 32, 32).astype(np.float32))


# revision 4
# speedup vs baseline: 1.6558x; 1.6558x over previous
"""Trainium2 Bass kernel for nn_AttentionHead_6786048328376.

8-head spatial attention block: q/k/v 1x1-conv projections with additive
positional embedding on q/k, softmax over the QUERY axis (dim=2), attention
apply, channel-major output, 2-layer 1x1-conv MLP with mish, residual add.

Sharding: pure data-parallel over batch — 8 batch elements, one per
NeuronCore. Weights are replicated; no collectives.

Per-core layout choices (C=512, N=H*W=1024, 8 heads, dh=32, ch=64):
  - x is held channel-major [512, 1024]; q/k land head-stacked [256, 1024]
    (row = 32*head + d) so each 128-partition tile holds 4 heads on the four
    32-row PE array strips -> scores use 4-way row-tiled K=32 matmuls.
  - scores are computed TRANSPOSED: sT[m, n] (key-major) so the softmax
    reduction over the query axis n is a free-axis reduction; exp runs on the
    scalar engine with fused row-sum accumulation (no max subtraction needed:
    |scores| <= ~8 for this problem scale).
  - v is computed directly transposed vT[n, c] = x.T @ Wv.T; the softmax
    1/sum is folded into vT rows (64 els/row) instead of dividing the 1M-el
    score matrix.
  - attention apply uses 2-way col-tiled matmuls (two heads concurrent,
    M=64 each) producing attn channel-major [512, 1024] with no transposes.
  - mish(x) = x*tanh(ln(1+exp(x))) via Exp -> Ln(bias=1) -> Tanh on ACT plus
    one fused (psum+b1)*tanh vector op.
"""

import numpy as np

_CACHE = {}


def _build():
    import concourse.bacc as bacc
    import concourse.tile as tile
    import concourse.mybir as mybir

    dt = mybir.dt
    F32 = dt.float32
    F32R = dt.float32r
    BF16 = dt.bfloat16

    def r(ap):
        # fp32 matmuls lower to 2 half-rate passes on the PE; float32r
        # streams at full rate (1 cycle/row for N>=256). The BIR verifier
        # requires f32r matmul operands to be PRODUCED as f32r, so the
        # operand tiles and their DRAM sources are typed float32r; this
        # helper is now a no-op marker at the matmul call sites.
        return ap
    Act = mybir.ActivationFunctionType
    Alu = mybir.AluOpType

    nc = bacc.Bacc("TRN2", target_bir_lowering=False, debug=False)

    x_d = nc.dram_tensor("x", [512, 1024], F32R, kind="ExternalInput").ap()
    wqkt_d = nc.dram_tensor("wqkt", [512, 512], F32R, kind="ExternalInput").ap()
    peb_d = nc.dram_tensor("peb", [4, 128, 1024], F32, kind="ExternalInput").ap()
    wvt_d = nc.dram_tensor("wvt", [512, 512], F32R, kind="ExternalInput").ap()
    bvb_d = nc.dram_tensor("bvb", [128, 512], F32, kind="ExternalInput").ap()
    w1t_d = nc.dram_tensor("w1t", [512, 512], F32R, kind="ExternalInput").ap()
    w2t_d = nc.dram_tensor("w2t", [512, 512], F32R, kind="ExternalInput").ap()
    b1_d = nc.dram_tensor("b1", [512, 1], F32, kind="ExternalInput").ap()
    b2_d = nc.dram_tensor("b2", [512, 1], F32, kind="ExternalInput").ap()
    out_d = nc.dram_tensor("out", [512, 1024], F32, kind="ExternalOutput").ap()

    with tile.TileContext(nc) as tc:
        with tc.tile_pool(name="persist", bufs=1) as per, \
             tc.tile_pool(name="mtmp", bufs=18) as mt, \
             tc.tile_pool(name="etp", bufs=16) as etp, \
             tc.tile_pool(name="small", bufs=8) as sm, \
             tc.tile_pool(name="ps", bufs=4, space="PSUM") as ps, \
             tc.tile_pool(name="av", bufs=4, space="PSUM") as av:

            def ptile(shape, dtype, name):
                return per.tile(shape, dtype, name=name, tag=name)

            x_sb = [ptile([128, 1024], F32R, f"xsb{i}") for i in range(4)]
            wqk_sb = [ptile([128, 512], F32R, f"wqk{i}") for i in range(4)]
            pe_sb = [ptile([128, 1024], F32, f"pe{i}") for i in range(4)]
            wv_sb = [ptile([128, 512], F32R, f"wv{i}") for i in range(4)]
            bv_sb = ptile([128, 512], F32, "bvsb")
            w1_sb = [ptile([128, 512], F32R, f"w1{i}") for i in range(4)]
            w2_sb = [ptile([128, 512], F32R, f"w2{i}") for i in range(4)]
            b1_sb = [ptile([128, 1], F32, f"b1c{i}") for i in range(4)]
            b2_sb = [ptile([128, 1], F32, f"b2c{i}") for i in range(4)]
            qk_sb = [ptile([128, 1024], F32R, f"qks{i}") for i in range(4)]
            vt_sb = [ptile([128, 512], F32, f"vts{i}") for i in range(8)]
            attn_sb = [ptile([128, 1024], F32R, f"attn{i}") for i in range(4)]
            mish_sb = [ptile([128, 1024], F32R, f"mish{i}") for i in range(4)]
            out_sb = [ptile([128, 1024], F32, f"osb{i}") for i in range(4)]

            dma = nc.sync.dma_start
            for i in range(4):
                dma(out=x_sb[i], in_=x_d[128 * i:128 * (i + 1), :])
            for i in range(4):
                dma(out=wqk_sb[i], in_=wqkt_d[128 * i:128 * (i + 1), :])
            for i in range(4):
                dma(out=pe_sb[i], in_=peb_d[i])
            for i in range(4):
                dma(out=wv_sb[i], in_=wvt_d[128 * i:128 * (i + 1), :])
            dma(out=bv_sb, in_=bvb_d)
            for i in range(4):
                dma(out=w1_sb[i], in_=w1t_d[128 * i:128 * (i + 1), :])
                dma(out=w2_sb[i], in_=w2t_d[128 * i:128 * (i + 1), :])
                dma(out=b1_sb[i], in_=b1_d[128 * i:128 * (i + 1), :])
                dma(out=b2_sb[i], in_=b2_d[128 * i:128 * (i + 1), :])

            mm = nc.tensor.matmul

            # q/k projections: qk[512, 1024] = WqkT.T @ x, then + (PE, bias)
            for t in range(4):
                for nh in range(2):
                    pt = ps.tile([128, 512], F32, name="sps", tag="sps")
                    for kc in range(4):
                        mm(pt, lhsT=r(wqk_sb[kc][:, 128 * t:128 * (t + 1)]),
                           rhs=r(x_sb[kc][:, 512 * nh:512 * (nh + 1)]),
                           start=(kc == 0), stop=(kc == 3))
                    nc.vector.tensor_add(
                        qk_sb[t][:, 512 * nh:512 * (nh + 1)], pt,
                        pe_sb[t][:, 512 * nh:512 * (nh + 1)])

            # vT[n, c] = x.T @ WvT, then + bv
            for i in range(8):
                pt = ps.tile([128, 512], F32, name="sps", tag="sps")
                for kc in range(4):
                    mm(pt, lhsT=r(x_sb[kc][:, 128 * i:128 * (i + 1)]),
                       rhs=r(wv_sb[kc]),
                       start=(kc == 0), stop=(kc == 3))
                nc.vector.tensor_add(vt_sb[i], pt, bv_sb)

            # attention: two head-groups of 4 heads
            for g in range(2):
                q_t = qk_sb[g]
                k_t = qk_sb[2 + g]
                av_t = {}
                for p_ in range(2):
                    for nh in range(2):
                        av_t[(p_, nh)] = av.tile([128, 512], F32,
                                                 name="avt", tag="av")
                for mc in range(8):
                    S = sm.tile([128, 8], F32, name="S", tag="S")
                    ets = {}
                    for nh in range(2):
                        for hp in range(4):
                            sp = ps.tile([128, 512], F32, name="sps", tag="sps")
                            mm(sp,
                               lhsT=r(k_t[32 * hp:32 * (hp + 1),
                                          128 * mc:128 * (mc + 1)]),
                               rhs=r(q_t[32 * hp:32 * (hp + 1),
                                         512 * nh:512 * (nh + 1)]),
                               start=True, stop=True,
                               tile_position=(32 * hp, 0))
                            et = etp.tile([128, 512], BF16, name="et", tag="et")
                            c = 4 * nh + hp
                            nc.scalar.activation(et, sp, Act.Exp,
                                                 accum_out=S[:, c:c + 1])
                            ets[(hp, nh)] = et
                    Ss = sm.tile([128, 4], F32, name="Ss", tag="Ss")
                    R = sm.tile([128, 4], F32, name="R", tag="R")
                    nc.vector.tensor_add(Ss, S[:, 0:4], S[:, 4:8])
                    nc.vector.reciprocal(R, Ss)
                    for hp in range(4):
                        h = 4 * g + hp
                        vts = sm.tile([128, 64], BF16, name="vtsc", tag="vtsc")
                        nc.vector.tensor_scalar_mul(
                            vts, vt_sb[mc][:, 64 * h:64 * (h + 1)],
                            R[:, hp:hp + 1])
                        p_, cp = hp // 2, 64 * (hp % 2)
                        for nh in range(2):
                            # two col-tiled accumulation series share each
                            # bank on disjoint partition halves; has_written
                            # is per-element so this is safe — the sim's
                            # coarse zero-region tracker is what we skip.
                            mm(av_t[(p_, nh)][cp:cp + 64, :], lhsT=vts,
                               rhs=ets[(hp, nh)],
                               start=(mc == 0), stop=(mc == 7),
                               tile_position=(0, cp), skip_group_check=True)
                for p_ in range(2):
                    for nh in range(2):
                        nc.vector.tensor_copy(
                            out=attn_sb[2 * g + p_][:, 512 * nh:512 * (nh + 1)],
                            in_=av_t[(p_, nh)])

            # MLP: h1 = W1 @ attn + b1; mish; out = W2 @ mish + b2 + x
            h1ps, t_t, sp_t, th_t = {}, {}, {}, {}
            for nh in range(2):
                pool, tg = (ps, "sps") if nh == 0 else (av, "av")
                for i in range(4):
                    pt = pool.tile([128, 512], F32, name="h1ps", tag=tg)
                    for kc in range(4):
                        mm(pt, lhsT=r(w1_sb[kc][:, 128 * i:128 * (i + 1)]),
                           rhs=r(attn_sb[kc][:, 512 * nh:512 * (nh + 1)]),
                           start=(kc == 0), stop=(kc == 3))
                    h1ps[(nh, i)] = pt
                    tt = mt.tile([128, 512], F32, name="mtt", tag="mtt")
                    nc.scalar.activation(tt, pt, Act.Exp, bias=b1_sb[i])
                    t_t[(nh, i)] = tt
            for key in list(t_t):
                spt = mt.tile([128, 512], F32, name="mtt", tag="mtt")
                nc.scalar.activation(spt, t_t[key], Act.Ln, bias=1.0)
                sp_t[key] = spt
            for key in list(sp_t):
                tht = mt.tile([128, 512], F32, name="mtt", tag="mtt")
                nc.scalar.activation(tht, sp_t[key], Act.Tanh)
                th_t[key] = tht
            for (nh, i), tht in th_t.items():
                nc.vector.scalar_tensor_tensor(
                    out=mish_sb[i][:, 512 * nh:512 * (nh + 1)],
                    in0=h1ps[(nh, i)], scalar=b1_sb[i], in1=tht,
                    op0=Alu.add, op1=Alu.mult)
            for nh in range(2):
                pool, tg = (ps, "sps") if nh == 0 else (av, "av")
                for j in range(4):
                    pt = pool.tile([128, 512], F32, name="h2ps", tag=tg)
                    for kc in range(4):
                        mm(pt, lhsT=r(w2_sb[kc][:, 128 * j:128 * (j + 1)]),
                           rhs=r(mish_sb[kc][:, 512 * nh:512 * (nh + 1)]),
                           start=(kc == 0), stop=(kc == 3))
                    nc.vector.scalar_tensor_tensor(
                        out=out_sb[j][:, 512 * nh:512 * (nh + 1)],
                        in0=pt, scalar=b2_sb[j],
                        in1=x_sb[j][:, 512 * nh:512 * (nh + 1)],
                        op0=Alu.add, op1=Alu.add)
            for j in range(4):
                dma(out=out_d[128 * j:128 * (j + 1), :], in_=out_sb[j])

    nc.compile()
    return nc


def _get_nc():
    if "nc" not in _CACHE:
        _CACHE["nc"] = _build()
    return _CACHE["nc"]


def _make_in_maps(inputs):
    x = np.asarray(inputs["x"], np.float32)
    PE = np.asarray(inputs["PE"], np.float32)
    Wq = np.asarray(inputs["Wq"], np.float32)
    bq = np.asarray(inputs["bq"], np.float32)
    Wk = np.asarray(inputs["Wk"], np.float32)
    bk = np.asarray(inputs["bk"], np.float32)
    Wv = np.asarray(inputs["Wv"], np.float32)
    bv = np.asarray(inputs["bv"], np.float32)
    W1 = np.asarray(inputs["W1"], np.float32)
    b1 = np.asarray(inputs["b1"], np.float32)
    W2 = np.asarray(inputs["W2"], np.float32)
    b2 = np.asarray(inputs["b2"], np.float32)

    s = np.float32(1.0 / np.sqrt(np.float32(32.0)))
    pef = PE.reshape(32, 1024)
    pe4 = np.tile(pef, (4, 1))  # [128, 1024], row = 32*j + d
    peb = np.stack([
        s * (pe4 + bq[0:128][:, None]),
        s * (pe4 + bq[128:256][:, None]),
        pe4 + bk[0:128][:, None],
        pe4 + bk[128:256][:, None],
    ]).astype(np.float32)
    wqkt = np.ascontiguousarray(
        np.concatenate([s * Wq, Wk], axis=0).T.astype(np.float32))
    wvt = np.ascontiguousarray(Wv.T.astype(np.float32))
    bvb = np.ascontiguousarray(
        np.broadcast_to(bv, (128, 512)).astype(np.float32))
    w1t = np.ascontiguousarray(W1.T.astype(np.float32))
    w2t = np.ascontiguousarray(W2.T.astype(np.float32))
    b1c = np.ascontiguousarray(b1.reshape(512, 1).astype(np.float32))
    b2c = np.ascontiguousarray(b2.reshape(512, 1).astype(np.float32))

    xb = np.ascontiguousarray(x.reshape(8, 512, 1024))
    shared = dict(wqkt=wqkt, peb=peb, wvt=wvt, bvb=bvb,
                  w1t=w1t, w2t=w2t, b1=b1c, b2=b2c)
    return [dict(x=np.ascontiguousarray(xb[i]), **shared) for i in range(8)]


def _run(in_maps, trace=False, **kwargs):
    from concourse import bass_utils
    nc = _get_nc()
    return bass_utils.run_bass_kernel_spmd(
        nc, in_maps, core_ids=list(range(8)), trace=trace, **kwargs)


def kernel(**inputs):
    in_maps = _make_in_maps(inputs)
    res = _run(in_maps)
    out = np.stack([r["out"] for r in res.results], axis=0)
    return np.ascontiguousarray(out.reshape(8, 512, 32, 32).astype(np.float32))


# revision 5
# speedup vs baseline: 1.7452x; 1.0540x over previous
"""Trainium2 Bass kernel for nn_AttentionHead_6786048328376.

8-head spatial attention block: q/k/v 1x1-conv projections with additive
positional embedding on q/k, softmax over the QUERY axis (dim=2), attention
apply, channel-major output, 2-layer 1x1-conv MLP with mish, residual add.

Sharding: pure data-parallel over batch — 8 batch elements, one per
NeuronCore. Weights are replicated; no collectives.

Per-core design (C=512, N=H*W=1024, 8 heads, dh=32, ch=64):
  - x is held channel-major [512, 1024]; q/k land head-stacked [256, 1024]
    (row = 32*head + d) so head-pairs sit on 32-row PE array strips ->
    scores use 2-way row-tiled K=32 matmuls (tile_position).
  - scores are computed TRANSPOSED: sT[m, n] (key-major) so the softmax
    reduction over the query axis n is a free-axis reduction. Both n-halves
    of one head land in one 2-bank psum tile, so exp is a single [128,1024]
    scalar-engine pass per (head, m-chunk) writing bf16 (no max subtraction
    needed: |scores| <= ~8 at this problem's scale).
  - row-sums of exp are split between the scalar engine (fused accum_out)
    and the vector engine (tensor_reduce over the bf16 exp tile) to balance
    the two engines; the scalar engine is the kernel bottleneck.
  - v is computed directly transposed vT[n, c] = x.T @ Wv.T; the softmax
    1/sum is folded into vT rows (64 els/row) instead of dividing the 1M-el
    score matrix.
  - attention apply uses 2-way col-tiled bf16 matmuls (both heads of the
    pair concurrent, M=64) accumulating over m-chunks, producing attn
    channel-major [512, 1024] with no transposes.
  - all fp32 matmuls use float32r (fp32 lowers to 2 half-rate PE passes;
    f32r streams 1 row/cycle). The BIR verifier requires f32r operands to
    be produced as f32r, so those tiles/DRAM tensors are typed float32r.
  - mish(x) = x*tanh(ln(1+exp(x))) via Exp -> Ln(bias=1) -> Tanh on the
    scalar engine (phased to avoid activation-table thrash) plus vector ops.
"""

import numpy as np

_CACHE = {}

# of the 8 m-chunks per (head, pair-group), how many use ACT accum_out for
# the exp row-sum; the rest use a DVE tensor_reduce over the bf16 exp tile.
ACT_ACCUM_PER_8 = 3


def _build():
    import concourse.bacc as bacc
    import concourse.tile as tile
    import concourse.mybir as mybir

    dt = mybir.dt
    F32 = dt.float32
    F32R = dt.float32r
    BF16 = dt.bfloat16
    Act = mybir.ActivationFunctionType
    Alu = mybir.AluOpType
    AxX = mybir.AxisListType.X

    nc = bacc.Bacc("TRN2", target_bir_lowering=False, debug=False)

    x_d = nc.dram_tensor("x", [512, 1024], F32R, kind="ExternalInput").ap()
    wqkt_d = nc.dram_tensor("wqkt", [512, 512], F32R, kind="ExternalInput").ap()
    peb_d = nc.dram_tensor("peb", [4, 128, 1024], F32, kind="ExternalInput").ap()
    wvt_d = nc.dram_tensor("wvt", [512, 512], F32R, kind="ExternalInput").ap()
    bvb_d = nc.dram_tensor("bvb", [128, 512], F32, kind="ExternalInput").ap()
    w1t_d = nc.dram_tensor("w1t", [512, 512], F32R, kind="ExternalInput").ap()
    w2t_d = nc.dram_tensor("w2t", [512, 512], F32R, kind="ExternalInput").ap()
    b1_d = nc.dram_tensor("b1", [512, 1], F32, kind="ExternalInput").ap()
    b2_d = nc.dram_tensor("b2", [512, 1], F32, kind="ExternalInput").ap()
    out_d = nc.dram_tensor("out", [512, 1024], F32, kind="ExternalOutput").ap()

    with tile.TileContext(nc) as tc:
        with tc.tile_pool(name="persist", bufs=1) as per, \
             tc.tile_pool(name="mtmp", bufs=18) as mt, \
             tc.tile_pool(name="etp", bufs=6) as etp, \
             tc.tile_pool(name="small", bufs=8) as sm, \
             tc.tile_pool(name="sbig", bufs=3, space="PSUM") as ps, \
             tc.tile_pool(name="av", bufs=2, space="PSUM") as av:

            def ptile(shape, dtype, name):
                return per.tile(shape, dtype, name=name, tag=name)

            x_sb = [ptile([128, 1024], F32R, f"xsb{i}") for i in range(4)]
            wqk_sb = [ptile([128, 512], F32R, f"wqk{i}") for i in range(4)]
            pe_sb = [ptile([128, 1024], F32, f"pe{i}") for i in range(4)]
            wv_sb = [ptile([128, 512], F32R, f"wv{i}") for i in range(4)]
            bv_sb = ptile([128, 512], F32, "bvsb")
            w1_sb = [ptile([128, 512], F32R, f"w1{i}") for i in range(4)]
            w2_sb = [ptile([128, 512], F32R, f"w2{i}") for i in range(4)]
            b1_sb = [ptile([128, 1], F32, f"b1c{i}") for i in range(4)]
            b2_sb = [ptile([128, 1], F32, f"b2c{i}") for i in range(4)]
            qk_sb = [ptile([128, 1024], F32R, f"qks{i}") for i in range(4)]
            vt_sb = [ptile([128, 512], F32, f"vts{i}") for i in range(8)]
            attn_sb = [ptile([128, 1024], F32R, f"attn{i}") for i in range(4)]
            mish_sb = [ptile([128, 1024], F32R, f"mish{i}") for i in range(4)]
            out_sb = [ptile([128, 1024], F32, f"osb{i}") for i in range(4)]

            dma = nc.sync.dma_start
            for i in range(4):
                dma(out=x_sb[i], in_=x_d[128 * i:128 * (i + 1), :])
            for i in range(4):
                dma(out=wqk_sb[i], in_=wqkt_d[128 * i:128 * (i + 1), :])
            for i in range(4):
                dma(out=pe_sb[i], in_=peb_d[i])
            for i in range(4):
                dma(out=wv_sb[i], in_=wvt_d[128 * i:128 * (i + 1), :])
            dma(out=bv_sb, in_=bvb_d)
            for i in range(4):
                dma(out=w1_sb[i], in_=w1t_d[128 * i:128 * (i + 1), :])
                dma(out=w2_sb[i], in_=w2t_d[128 * i:128 * (i + 1), :])
                dma(out=b1_sb[i], in_=b1_d[128 * i:128 * (i + 1), :])
                dma(out=b2_sb[i], in_=b2_d[128 * i:128 * (i + 1), :])

            mm = nc.tensor.matmul

            # q/k projections: qk[512, 1024] = WqkT.T @ x, then + (PE, bias)
            for t in range(4):
                for nh in range(2):
                    pt = ps.tile([128, 512], F32, name="pps", tag="sbig")
                    for kc in range(4):
                        mm(pt, lhsT=wqk_sb[kc][:, 128 * t:128 * (t + 1)],
                           rhs=x_sb[kc][:, 512 * nh:512 * (nh + 1)],
                           start=(kc == 0), stop=(kc == 3))
                    nc.vector.tensor_add(
                        qk_sb[t][:, 512 * nh:512 * (nh + 1)], pt,
                        pe_sb[t][:, 512 * nh:512 * (nh + 1)])

            # vT[n, c] = x.T @ WvT, then + bv
            for i in range(8):
                pt = ps.tile([128, 512], F32, name="pps", tag="sbig")
                for kc in range(4):
                    mm(pt, lhsT=x_sb[kc][:, 128 * i:128 * (i + 1)],
                       rhs=wv_sb[kc],
                       start=(kc == 0), stop=(kc == 3))
                nc.vector.tensor_add(vt_sb[i], pt, bv_sb)

            # attention: four head-pair groups
            for pg in range(4):
                g = pg // 2           # which 128-row q/k tile
                off0 = 64 * (pg % 2)  # partition offset of this pair in it
                q_t = qk_sb[g]
                k_t = qk_sb[2 + g]
                avt = [av.tile([128, 512], F32, name="avt", tag="av")
                       for _ in range(2)]  # [nh]
                for mc in range(8):
                    S = sm.tile([128, 2], F32, name="S", tag="S")
                    R = sm.tile([128, 2], F32, name="R", tag="R")
                    ebig = []
                    for hp in range(2):
                        off = off0 + 32 * hp
                        sp = ps.tile([128, 1024], F32, name="sps", tag="sbig")
                        for nh in range(2):
                            mm(sp[:, 512 * nh:512 * (nh + 1)],
                               lhsT=k_t[off:off + 32, 128 * mc:128 * (mc + 1)],
                               rhs=q_t[off:off + 32, 512 * nh:512 * (nh + 1)],
                               start=True, stop=True,
                               tile_position=(off, 0))
                        et = etp.tile([128, 1024], BF16, name="et", tag="et")
                        if mc % 8 < ACT_ACCUM_PER_8:
                            nc.scalar.activation(et, sp, Act.Exp,
                                                 accum_out=S[:, hp:hp + 1])
                        else:
                            nc.scalar.activation(et, sp, Act.Exp)
                            nc.vector.tensor_reduce(
                                S[:, hp:hp + 1], et, axis=AxX, op=Alu.add)
                        ebig.append(et)
                    nc.vector.reciprocal(R, S)
                    for hp in range(2):
                        h = 2 * pg + hp
                        vts = sm.tile([128, 64], BF16, name="vtsc", tag="vtsc")
                        nc.vector.tensor_scalar_mul(
                            vts, vt_sb[mc][:, 64 * h:64 * (h + 1)],
                            R[:, hp:hp + 1])
                        for nh in range(2):
                            # two col-tiled accumulation series share each
                            # bank on disjoint partition halves; has_written
                            # is per-element so this is safe — the sim's
                            # coarse zero-region tracker is what we skip.
                            mm(avt[nh][64 * hp:64 * hp + 64, :], lhsT=vts,
                               rhs=ebig[hp][:, 512 * nh:512 * (nh + 1)],
                               start=(mc == 0), stop=(mc == 7),
                               tile_position=(0, 64 * hp),
                               skip_group_check=True)
                for nh in range(2):
                    nc.vector.tensor_copy(
                        out=attn_sb[pg][:, 512 * nh:512 * (nh + 1)],
                        in_=avt[nh])

            # MLP: h1 = W1 @ attn + b1; mish; out = W2 @ mish + b2 + x
            h1f, t_t, sp_t, th_t = {}, {}, {}, {}
            for nh in range(2):
                for i in range(4):
                    pt = av.tile([128, 512], F32, name="h1ps", tag="av")
                    for kc in range(4):
                        mm(pt, lhsT=w1_sb[kc][:, 128 * i:128 * (i + 1)],
                           rhs=attn_sb[kc][:, 512 * nh:512 * (nh + 1)],
                           start=(kc == 0), stop=(kc == 3))
                    tt = mt.tile([128, 512], F32, name="mtt", tag="mtt")
                    nc.scalar.activation(tt, pt, Act.Exp, bias=b1_sb[i])
                    t_t[(nh, i)] = tt
                    hf = mt.tile([128, 512], F32, name="mtt", tag="mtt")
                    nc.vector.tensor_scalar_add(hf, pt, b1_sb[i])
                    h1f[(nh, i)] = hf
            for key in list(t_t):
                spt = mt.tile([128, 512], F32, name="mtt", tag="mtt")
                nc.scalar.activation(spt, t_t[key], Act.Ln, bias=1.0)
                sp_t[key] = spt
            for key in list(sp_t):
                tht = mt.tile([128, 512], F32, name="mtt", tag="mtt")
                nc.scalar.activation(tht, sp_t[key], Act.Tanh)
                th_t[key] = tht
            for (nh, i), tht in th_t.items():
                nc.vector.tensor_mul(
                    mish_sb[i][:, 512 * nh:512 * (nh + 1)],
                    h1f[(nh, i)], tht)
            for nh in range(2):
                for j in range(4):
                    pt = av.tile([128, 512], F32, name="h2ps", tag="av")
                    for kc in range(4):
                        mm(pt, lhsT=w2_sb[kc][:, 128 * j:128 * (j + 1)],
                           rhs=mish_sb[kc][:, 512 * nh:512 * (nh + 1)],
                           start=(kc == 0), stop=(kc == 3))
                    nc.vector.scalar_tensor_tensor(
                        out=out_sb[j][:, 512 * nh:512 * (nh + 1)],
                        in0=pt, scalar=b2_sb[j],
                        in1=x_sb[j][:, 512 * nh:512 * (nh + 1)],
                        op0=Alu.add, op1=Alu.add)
            for j in range(4):
                dma(out=out_d[128 * j:128 * (j + 1), :], in_=out_sb[j])

    nc.compile()
    return nc


def _get_nc():
    if "nc" not in _CACHE:
        _CACHE["nc"] = _build()
    return _CACHE["nc"]


def _make_in_maps(inputs):
    x = np.asarray(inputs["x"], np.float32)
    PE = np.asarray(inputs["PE"], np.float32)
    Wq = np.asarray(inputs["Wq"], np.float32)
    bq = np.asarray(inputs["bq"], np.float32)
    Wk = np.asarray(inputs["Wk"], np.float32)
    bk = np.asarray(inputs["bk"], np.float32)
    Wv = np.asarray(inputs["Wv"], np.float32)
    bv = np.asarray(inputs["bv"], np.float32)
    W1 = np.asarray(inputs["W1"], np.float32)
    b1 = np.asarray(inputs["b1"], np.float32)
    W2 = np.asarray(inputs["W2"], np.float32)
    b2 = np.asarray(inputs["b2"], np.float32)

    s = np.float32(1.0 / np.sqrt(np.float32(32.0)))
    pef = PE.reshape(32, 1024)
    pe4 = np.tile(pef, (4, 1))  # [128, 1024], row = 32*j + d
    peb = np.stack([
        s * (pe4 + bq[0:128][:, None]),
        s * (pe4 + bq[128:256][:, None]),
        pe4 + bk[0:128][:, None],
        pe4 + bk[128:256][:, None],
    ]).astype(np.float32)
    wqkt = np.ascontiguousarray(
        np.concatenate([s * Wq, Wk], axis=0).T.astype(np.float32))
    wvt = np.ascontiguousarray(Wv.T.astype(np.float32))
    bvb = np.ascontiguousarray(
        np.broadcast_to(bv, (128, 512)).astype(np.float32))
    w1t = np.ascontiguousarray(W1.T.astype(np.float32))
    w2t = np.ascontiguousarray(W2.T.astype(np.float32))
    b1c = np.ascontiguousarray(b1.reshape(512, 1).astype(np.float32))
    b2c = np.ascontiguousarray(b2.reshape(512, 1).astype(np.float32))

    xb = np.ascontiguousarray(x.reshape(8, 512, 1024))
    shared = dict(wqkt=wqkt, peb=peb, wvt=wvt, bvb=bvb,
                  w1t=w1t, w2t=w2t, b1=b1c, b2=b2c)
    return [dict(x=np.ascontiguousarray(xb[i]), **shared) for i in range(8)]


def _run(in_maps, trace=False, **kwargs):
    from concourse import bass_utils
    nc = _get_nc()
    return bass_utils.run_bass_kernel_spmd(
        nc, in_maps, core_ids=list(range(8)), trace=trace, **kwargs)


def kernel(**inputs):
    in_maps = _make_in_maps(inputs)
    res = _run(in_maps)
    out = np.stack([r["out"] for r in res.results], axis=0)
    return np.ascontiguousarray(out.reshape(8, 512, 32, 32).astype(np.float32))
